# revision 12
# baseline (speedup 1.0000x reference)
"""Trainium2 Bass kernel for nn_DynamicConv (dense_cnn).

out[i, j, co, h, w] = sum_k (conv_k(x_i)[co, h, w] + b_k[co]) * attn[j, k]
attn = softmax(softmax(MLP(meanpool(x)), k) / TAU, k)

Sharding: data-parallel over batch i across 8 cores.  Each core convolves its
own sample (9 shifted matmuls over a zero-padded image, contraction = CIN=128,
fp32r) and computes the full [B, K] attention matrix locally from a replicated
copy of x (it is tiny), then applies the cross-batch blend as one
block-diagonal matmul per 16-channel group:
  contraction 64 = (k=4) x (co16), M = 128 = (j=8) x (co16).
Conv weights are host-packed so output channels land in (co, k)-interleaved
partition order, which makes the blend's rhs a contiguous partition range.
All matmul operands are float32r (FP22 multiply, fp32 accumulate) — full PE
rate; the BIR verifier requires producers of those tiles to emit float32r.
"""

import sys

import numpy as np

if "/opt/trn_rl_repo" not in sys.path:
    sys.path.insert(0, "/opt/trn_rl_repo")

import concourse.bacc as bacc
import concourse.bass as bass
import concourse.mybir as mybir
import concourse.tile as tile

F32 = mybir.dt.float32
F32R = mybir.dt.float32r
AF = mybir.ActivationFunctionType
AX = mybir.AxisListType
ALU = mybir.AluOpType

B = 8
CIN = 128
COUT = 256
K = 4
KS = 3
HW = 48
HW2 = HW * HW          # 2304
WP = HW + 2            # 50 (padded)
HID = 256
TAU = 30.0
NCORES = 8

ROW_GROUPS = [(0, 10), (10, 10), (20, 10), (30, 10), (40, 8)]
CHUNKS = [(0, 512), (512, 512), (1024, 512), (1536, 512), (2048, 256)]


def build_nc():
    nc = bacc.Bacc("TRN2", debug=False)

    x_all = nc.dram_tensor("x_all", [B, CIN, HW2], F32, kind="ExternalInput").ap()
    xi = nc.dram_tensor("xi", [CIN, HW2], F32R, kind="ExternalInput").ap()
    # [ci, t, tap, p] flattened; p = c*4 + k encodes (co = 32 t + c, k)
    wconv = nc.dram_tensor(
        "wconv", [CIN, 8 * 9 * 128], F32R, kind="ExternalInput"
    ).ap()
    bconv = nc.dram_tensor("bconv", [128, 8], F32, kind="ExternalInput").ap()
    w1t = nc.dram_tensor("w1t", [CIN, HID], F32R, kind="ExternalInput").ap()
    b1c = nc.dram_tensor("b1c", [128, 2], F32, kind="ExternalInput").ap()
    w2t = nc.dram_tensor("w2t", [128, 2 * K], F32R, kind="ExternalInput").ap()
    b2r = nc.dram_tensor("b2r", [1, K], F32R, kind="ExternalInput").ap()
    ident8 = nc.dram_tensor("ident8", [B, B], F32R, kind="ExternalInput").ap()
    # memset can't write float32r tiles (walrus ISA check) — ship constants
    zer128 = nc.dram_tensor("zer128", [128, 128], F32R, kind="ExternalInput").ap()
    one18 = nc.dram_tensor("one18", [1, B], F32R, kind="ExternalInput").ap()
    out = nc.dram_tensor("out", [B, COUT, HW2], F32, kind="ExternalOutput").ap()

    with tile.TileContext(nc) as tc:
        with (
            tc.tile_pool(name="const", bufs=1) as const,
            tc.tile_pool(name="xpool", bufs=3) as xpool,
            tc.tile_pool(name="csb", bufs=3) as csb_pool,
            tc.tile_pool(name="osb", bufs=3) as osb_pool,
            tc.tile_pool(name="psA", bufs=4, space="PSUM") as psA,
            tc.tile_pool(name="psB", bufs=3, space="PSUM") as psB,
            tc.tile_pool(name="psM", bufs=1, space="PSUM") as psM,
        ):
            # ---- constants / weights ----
            wt = []
            for t in range(8):
                w = const.tile([128, 9 * 128], F32R, tag=f"wt{t}")
                nc.sync.dma_start(w[:], wconv[:, t * 9 * 128 : (t + 1) * 9 * 128])
                wt.append(w)
            bct = const.tile([128, 8], F32)
            nc.sync.dma_start(bct[:], bconv[:, :])
            w1s = const.tile([128, HID], F32R)
            nc.sync.dma_start(w1s[:], w1t[:, :])
            b1s = const.tile([128, 2], F32)
            nc.sync.dma_start(b1s[:], b1c[:, :])
            w2s = const.tile([128, 2 * K], F32R)
            nc.sync.dma_start(w2s[:], w2t[:, :])
            b2s = const.tile([1, K], F32R)
            nc.sync.dma_start(b2s[:], b2r[:, :])
            id8 = const.tile([B, B], F32R)
            nc.sync.dma_start(id8[:], ident8[:, :])
            ones = const.tile([1, B], F32R)
            nc.sync.dma_start(ones[:], one18[:, :])

            # ---- padded own-sample image: interior from xi, border zeros ----
            xp = const.tile([128, WP * WP], F32R)
            xp3 = xp[:].rearrange("p (h w) -> p h w", w=WP)
            nc.sync.dma_start(xp3[:, 0, 0:WP], zer128[:, 0:WP])
            nc.sync.dma_start(xp3[:, WP - 1, 0:WP], zer128[:, 0:WP])
            nc.sync.dma_start(xp3[:, 1 : 1 + HW, 0], zer128[:, 0:HW])
            nc.sync.dma_start(xp3[:, 1 : 1 + HW, WP - 1], zer128[:, 0:HW])
            nc.sync.dma_start(xp3[:, 1 : 1 + HW, 1 : 1 + HW], xi[:, :])

            # ---- global average pooling of every sample (for attention) ----
            pooledT = const.tile([128, B], F32R)  # [ci, j] sums; 1/HW2 is in w1t
            for j in range(B):
                xt = xpool.tile([128, HW2], F32, tag="xt")
                nc.sync.dma_start(xt[:], x_all[j])
                with nc.allow_low_precision(reason="fp32r matmul operand"):
                    nc.vector.tensor_reduce(
                        pooledT[:, j : j + 1], xt[:], axis=AX.X, op=ALU.add
                    )

            cs_tiles = [None] * 8

            def emit_conv(t):
                cs = csb_pool.tile([128, HW2], F32R, tag="csb")
                cs_tiles[t] = cs
                for (r0, R) in ROW_GROUPS:
                    pt = psA.tile([128, R * HW], F32, tag="cps")
                    for tap in range(9):
                        dh, dw = divmod(tap, 3)
                        rhs = xp3[:, r0 + dh : r0 + dh + R, dw : dw + HW]
                        nc.tensor.matmul(
                            pt[:],
                            lhsT=wt[t][:, tap * 128 : (tap + 1) * 128],
                            rhs=rhs,
                            start=(tap == 0),
                            stop=(tap == 8),
                        )
                    # PSUM -> SBUF eviction, fused with the conv bias add
                    nc.scalar.activation(
                        cs[:, r0 * HW : (r0 + R) * HW],
                        pt[:],
                        AF.Identity,
                        bias=bct[:, t : t + 1],
                    )

            def emit_blend(t, BD):
                cs = cs_tiles[t]
                for u in range(2):
                    g = 2 * t + u
                    ob = osb_pool.tile([128, HW2], F32, tag="osb")
                    for (c0, C) in CHUNKS:
                        bp = psB.tile([128, C], F32, tag="bps")
                        nc.tensor.matmul(
                            bp[:],
                            lhsT=BD[64 * u : 64 * u + 64, :],
                            rhs=cs[64 * u : 64 * u + 64, c0 : c0 + C],
                            start=True,
                            stop=True,
                        )
                        nc.vector.tensor_copy(ob[:, c0 : c0 + C], bp[:])
                    nc.sync.dma_start(out[:, 16 * g : 16 * g + 16, :], ob[:])

            # conv for t=0,1 first so the PE has dense work while pooling DMAs run
            emit_conv(0)
            emit_conv(1)

            # ---- attention MLP + double softmax ----
            hd = []
            for h in range(2):
                hps = psM.tile([128, B], F32, tag="mlp")
                nc.tensor.matmul(
                    hps[:],
                    lhsT=w1s[:, h * 128 : (h + 1) * 128],
                    rhs=pooledT[:],
                    start=True,
                    stop=True,
                )
                hsb = const.tile([128, B], F32R, tag=f"hd{h}")
                nc.scalar.activation(hsb[:], hps[:], AF.Relu, bias=b1s[:, h : h + 1])
                hd.append(hsb)

            lps = psM.tile([B, K], F32, tag="mlp")
            nc.tensor.matmul(
                lps[:], lhsT=hd[0][:], rhs=w2s[:, 0:K], start=True, stop=False
            )
            nc.tensor.matmul(
                lps[:], lhsT=hd[1][:], rhs=w2s[:, K : 2 * K], start=False, stop=False
            )
            nc.tensor.matmul(
                lps[:], lhsT=ones[:], rhs=b2s[:], start=False, stop=True
            )

            mxn = const.tile([B, 1], F32)
            nc.vector.tensor_reduce(mxn[:], lps[:], axis=AX.X, op=ALU.max, negate=True)
            e1 = const.tile([B, K], F32)
            nc.scalar.activation(e1[:], lps[:], AF.Exp, bias=mxn[:, 0:1], scale=1.0)
            s1 = const.tile([B, 1], F32)
            nc.vector.tensor_reduce(s1[:], e1[:], axis=AX.X, op=ALU.add)
            r1 = const.tile([B, 1], F32)
            nc.vector.reciprocal(r1[:], s1[:])
            a1 = const.tile([B, K], F32)
            nc.vector.tensor_scalar_mul(a1[:], e1[:], r1[:, 0:1])

            m2n = const.tile([B, 1], F32)
            nc.vector.tensor_reduce(m2n[:], a1[:], axis=AX.X, op=ALU.max, negate=True)
            m2ns = const.tile([B, 1], F32)
            nc.scalar.mul(m2ns[:], m2n[:], 1.0 / TAU)
            e2 = const.tile([B, K], F32)
            nc.scalar.activation(e2[:], a1[:], AF.Exp, bias=m2ns[:, 0:1], scale=1.0 / TAU)
            s2 = const.tile([B, 1], F32)
            nc.vector.tensor_reduce(s2[:], e2[:], axis=AX.X, op=ALU.add)
            r2 = const.tile([B, 1], F32)
            nc.vector.reciprocal(r2[:], s2[:])
            attn = const.tile([B, K], F32R)
            nc.vector.tensor_scalar_mul(attn[:], e2[:], r2[:, 0:1])

            # attn [j, k] -> attn_T [k, j] via PE transpose
            tps = psM.tile([K, B], F32R, tag="mlp")
            nc.tensor.transpose(tps[:], attn[:], id8[:])
            atT = const.tile([K, B], F32R)
            nc.scalar.copy(atT[:], tps[:])

            # block-diagonal blend weights: BD[c*4+k, j*16+c] = attn[j, k],
            # replicated in both partition halves so the lhsT slice's
            # base_partition can match the rhs slice's (matmul requirement)
            BD = const.tile([128, 128], F32R)
            nc.sync.dma_start(BD[:], zer128[:, :])
            BDv = BD[:].rearrange("p (j c) -> p j c", c=16)
            for half in range(2):
                for c in range(16):
                    # compute engines can't start at partition 4c; DMA can
                    p0 = 64 * half + c * 4
                    nc.sync.dma_start(BDv[p0 : p0 + 4, :, c], atT[:])

            # interleave remaining convs with blends so output DMA overlaps compute
            emit_blend(0, BD)
            emit_blend(1, BD)
            for t in range(2, 8):
                emit_conv(t)
                emit_blend(t, BD)

    nc.compile()
    return nc


def pack_inputs(x, conv_w, conv_b, w1, b1, w2, b2):
    """Host-side layout packing (no arithmetic beyond constant folding of the
    mean-pool scale into w1)."""
    x = np.ascontiguousarray(x, dtype=np.float32)
    x_all = x.reshape(B, CIN, HW2)

    # conv_w [K, COUT, CIN, 3, 3] -> [ci, t, tap, p] with p = c*4 + k,
    # co = 32 t + c
    w = np.asarray(conv_w, dtype=np.float32).transpose(2, 3, 4, 0, 1)  # ci kh kw k co
    w = w.reshape(CIN, KS, KS, K, 8, 32)  # ci kh kw k t c
    w = w.transpose(0, 4, 1, 2, 5, 3)  # ci t kh kw c k
    wconv = np.ascontiguousarray(w.reshape(CIN, 8 * 9 * 128))

    bc = np.asarray(conv_b, dtype=np.float32).reshape(K, 8, 32)  # k t c
    bconv = np.ascontiguousarray(bc.transpose(1, 2, 0).reshape(8, 128).T)  # [p, t]

    w1t = np.ascontiguousarray(np.asarray(w1, dtype=np.float32).T) / float(HW2)
    b1c = np.ascontiguousarray(np.asarray(b1, dtype=np.float32).reshape(2, 128).T)
    w2T = np.asarray(w2, dtype=np.float32).T  # [256, 4]
    w2t = np.ascontiguousarray(np.concatenate([w2T[:128], w2T[128:]], axis=1))
    b2r = np.asarray(b2, dtype=np.float32).reshape(1, K)
    ident8 = np.eye(B, dtype=np.float32)

    common = dict(
        x_all=x_all, wconv=wconv, bconv=bconv, w1t=w1t, b1c=b1c,
        w2t=w2t, b2r=b2r, ident8=ident8,
        zer128=np.zeros((128, 128), dtype=np.float32),
        one18=np.ones((1, B), dtype=np.float32),
    )
    in_maps = [dict(common, xi=np.ascontiguousarray(x_all[i])) for i in range(NCORES)]
    return in_maps


def run(inputs, trace=False):
    from concourse.bass_utils import run_bass_kernel_spmd

    nc = build_nc()
    in_maps = pack_inputs(**inputs)
    res = run_bass_kernel_spmd(
        nc, in_maps, core_ids=list(range(NCORES)), trace=trace
    )
    slabs = [res.results[i]["out"] for i in range(NCORES)]
    out = np.stack(slabs, axis=0).reshape(B, B, COUT, HW, HW)
    return out, res


def kernel(**inputs) -> np.ndarray:
    out, _ = run(inputs, trace=False)
    return out


# revision 19
# speedup vs baseline: 1.0780x; 1.0780x over previous
"""Trainium2 Bass kernel for nn_DynamicConv (dense_cnn).

out[i, j, co, h, w] = sum_k (conv_k(x_i)[co, h, w] + b_k[co]) * attn[j, k]
attn = softmax(softmax(MLP(meanpool(x)), k) / TAU, k)

Sharding: data-parallel over batch i across 8 cores.  Each core convolves its
own sample (9 shifted matmuls over a zero-padded image, contraction = CIN=128,
fp32r) and computes the full [B, K] attention matrix locally from a replicated
copy of x (it is tiny), then applies the cross-batch blend as one
block-diagonal matmul per 16-channel group:
  contraction 64 = (k=4) x (co16), M = 128 = (j=8) x (co16).
Conv weights are host-packed so output channels land in (co, k)-interleaved
partition order, which makes the blend's rhs a contiguous partition range.
All matmul operands are float32r (FP22 multiply, fp32 accumulate) — full PE
rate; the BIR verifier requires producers of those tiles to emit float32r.
"""

import sys

import numpy as np

if "/opt/trn_rl_repo" not in sys.path:
    sys.path.insert(0, "/opt/trn_rl_repo")

import concourse.bacc as bacc
import concourse.bass as bass
import concourse.mybir as mybir
import concourse.tile as tile

F32 = mybir.dt.float32
F32R = mybir.dt.float32r
AF = mybir.ActivationFunctionType
AX = mybir.AxisListType
ALU = mybir.AluOpType

B = 8
CIN = 128
COUT = 256
K = 4
KS = 3
HW = 48
HW2 = HW * HW          # 2304
WP = HW + 2            # 50 (padded)
HID = 256
TAU = 30.0
NCORES = 8

ROW_GROUPS = [(0, 10), (10, 10), (20, 10), (30, 10), (40, 8)]
CHUNKS = [(0, 512), (512, 512), (1024, 512), (1536, 512), (2048, 256)]


def build_nc():
    nc = bacc.Bacc("TRN2", debug=False)

    x_all = nc.dram_tensor("x_all", [B, CIN, HW2], F32, kind="ExternalInput").ap()
    xi = nc.dram_tensor("xi", [CIN, HW2], F32R, kind="ExternalInput").ap()
    # [ci, t, tap, p] flattened; p = c*4 + k encodes (co = 32 t + c, k)
    wconv = nc.dram_tensor(
        "wconv", [CIN, 8 * 9 * 128], F32R, kind="ExternalInput"
    ).ap()
    bconv = nc.dram_tensor("bconv", [128, 8], F32, kind="ExternalInput").ap()
    w1t = nc.dram_tensor("w1t", [CIN, HID], F32R, kind="ExternalInput").ap()
    b1c = nc.dram_tensor("b1c", [128, 2], F32, kind="ExternalInput").ap()
    w2t = nc.dram_tensor("w2t", [128, 2 * K], F32R, kind="ExternalInput").ap()
    b2r = nc.dram_tensor("b2r", [1, K], F32R, kind="ExternalInput").ap()
    ident8 = nc.dram_tensor("ident8", [B, B], F32R, kind="ExternalInput").ap()
    # memset can't write float32r tiles (walrus ISA check) — ship constants
    zer128 = nc.dram_tensor("zer128", [128, 128], F32R, kind="ExternalInput").ap()
    one18 = nc.dram_tensor("one18", [1, B], F32R, kind="ExternalInput").ap()
    out = nc.dram_tensor("out", [B, COUT, HW2], F32, kind="ExternalOutput").ap()

    with tile.TileContext(nc) as tc:
        with (
            tc.tile_pool(name="const", bufs=1) as const,
            tc.tile_pool(name="xpool", bufs=3) as xpool,
            tc.tile_pool(name="csb", bufs=4) as csb_pool,
            tc.tile_pool(name="osb", bufs=3) as osb_pool,
            tc.tile_pool(name="psA", bufs=3, space="PSUM") as psA,
            tc.tile_pool(name="psB", bufs=4, space="PSUM") as psB,
            tc.tile_pool(name="psM", bufs=1, space="PSUM") as psM,
        ):
            # ---- conv-critical loads first: image, then weights ----
            xfull = const.tile([128, HW2], F32R)
            nc.sync.dma_start(xfull[:], xi[:, :])
            ztile = const.tile([128, 128], F32R)
            nc.sync.dma_start(ztile[:], zer128[:, :])

            # padded image built on-chip (a strided DMA here would shatter
            # into 192B descriptors and swamp the queues)
            xp = const.tile([128, WP * WP], F32R)
            xp3 = xp[:].rearrange("p (h w) -> p h w", w=WP)
            xf3 = xfull[:].rearrange("p (h w) -> p h w", w=HW)
            nc.vector.tensor_copy(xp3[:, 1 : 1 + HW, 1 : 1 + HW], xf3[:, :, :])
            nc.vector.tensor_copy(xp3[:, 0, 0:WP], ztile[:, 0:WP])
            nc.vector.tensor_copy(xp3[:, WP - 1, 0:WP], ztile[:, 0:WP])
            nc.vector.tensor_copy(xp3[:, 1 : 1 + HW, 0], ztile[:, 0:HW])
            nc.vector.tensor_copy(xp3[:, 1 : 1 + HW, WP - 1], ztile[:, 0:HW])

            wt = []
            for t in range(8):
                w = const.tile([128, 9 * 128], F32R, tag=f"wt{t}")
                nc.sync.dma_start(w[:], wconv[:, t * 9 * 128 : (t + 1) * 9 * 128])
                wt.append(w)
            bct = const.tile([128, 8], F32)
            nc.sync.dma_start(bct[:], bconv[:, :])
            w1s = const.tile([128, HID], F32R)
            nc.sync.dma_start(w1s[:], w1t[:, :])
            b1s = const.tile([128, 2], F32)
            nc.sync.dma_start(b1s[:], b1c[:, :])
            w2s = const.tile([128, 2 * K], F32R)
            nc.sync.dma_start(w2s[:], w2t[:, :])
            b2s = const.tile([1, K], F32R)
            nc.sync.dma_start(b2s[:], b2r[:, :])
            id8 = const.tile([B, B], F32R)
            nc.sync.dma_start(id8[:], ident8[:, :])
            ones = const.tile([1, B], F32R)
            nc.sync.dma_start(ones[:], one18[:, :])

            # ---- global average pooling of every sample (for attention) ----
            pooledT = const.tile([128, B], F32R)  # [ci, j] sums; 1/HW2 is in w1t
            for j in range(B):
                xt = xpool.tile([128, HW2], F32, tag="xt")
                nc.sync.dma_start(xt[:], x_all[j])
                with nc.allow_low_precision(reason="fp32r matmul operand"):
                    nc.vector.tensor_reduce(
                        pooledT[:, j : j + 1], xt[:], axis=AX.X, op=ALU.add
                    )

            cs_tiles = [None] * 8

            def emit_conv(t):
                cs = csb_pool.tile([128, HW2], F32R, tag="csb")
                cs_tiles[t] = cs
                for (r0, R) in ROW_GROUPS:
                    pt = psA.tile([128, R * HW], F32, tag="cps")
                    for tap in range(9):
                        dh, dw = divmod(tap, 3)
                        rhs = xp3[:, r0 + dh : r0 + dh + R, dw : dw + HW]
                        nc.tensor.matmul(
                            pt[:],
                            lhsT=wt[t][:, tap * 128 : (tap + 1) * 128],
                            rhs=rhs,
                            start=(tap == 0),
                            stop=(tap == 8),
                        )
                    # PSUM -> SBUF eviction, fused with the conv bias add
                    nc.scalar.activation(
                        cs[:, r0 * HW : (r0 + R) * HW],
                        pt[:],
                        AF.Identity,
                        bias=bct[:, t : t + 1],
                    )

            def emit_blend(t, BD):
                cs = cs_tiles[t]
                for u in range(2):
                    g = 2 * t + u
                    ob = osb_pool.tile([128, HW2], F32, tag="osb")
                    for (c0, C) in CHUNKS:
                        bp = psB.tile([128, C], F32, tag="bps")
                        nc.tensor.matmul(
                            bp[:],
                            lhsT=BD[64 * u : 64 * u + 64, :],
                            rhs=cs[64 * u : 64 * u + 64, c0 : c0 + C],
                            start=True,
                            stop=True,
                        )
                        nc.vector.tensor_copy(ob[:, c0 : c0 + C], bp[:])
                    nc.sync.dma_start(out[:, 16 * g : 16 * g + 16, :], ob[:])

            # conv for t=0..2 first so the PE has dense work while pooling DMAs run
            emit_conv(0)
            emit_conv(1)
            emit_conv(2)

            # ---- attention MLP + double softmax ----
            hd = []
            for h in range(2):
                hps = psM.tile([128, B], F32, tag="mlp")
                nc.tensor.matmul(
                    hps[:],
                    lhsT=w1s[:, h * 128 : (h + 1) * 128],
                    rhs=pooledT[:],
                    start=True,
                    stop=True,
                )
                hsb = const.tile([128, B], F32R, tag=f"hd{h}")
                nc.scalar.activation(hsb[:], hps[:], AF.Relu, bias=b1s[:, h : h + 1])
                hd.append(hsb)

            lps = psM.tile([B, K], F32, tag="mlp")
            nc.tensor.matmul(
                lps[:], lhsT=hd[0][:], rhs=w2s[:, 0:K], start=True, stop=False
            )
            nc.tensor.matmul(
                lps[:], lhsT=hd[1][:], rhs=w2s[:, K : 2 * K], start=False, stop=False
            )
            nc.tensor.matmul(
                lps[:], lhsT=ones[:], rhs=b2s[:], start=False, stop=True
            )

            mxn = const.tile([B, 1], F32)
            nc.vector.tensor_reduce(mxn[:], lps[:], axis=AX.X, op=ALU.max, negate=True)
            e1 = const.tile([B, K], F32)
            nc.scalar.activation(e1[:], lps[:], AF.Exp, bias=mxn[:, 0:1], scale=1.0)
            s1 = const.tile([B, 1], F32)
            nc.vector.tensor_reduce(s1[:], e1[:], axis=AX.X, op=ALU.add)
            r1 = const.tile([B, 1], F32)
            nc.vector.reciprocal(r1[:], s1[:])
            a1 = const.tile([B, K], F32)
            nc.vector.tensor_scalar_mul(a1[:], e1[:], r1[:, 0:1])

            m2n = const.tile([B, 1], F32)
            nc.vector.tensor_reduce(m2n[:], a1[:], axis=AX.X, op=ALU.max, negate=True)
            m2ns = const.tile([B, 1], F32)
            nc.scalar.mul(m2ns[:], m2n[:], 1.0 / TAU)
            e2 = const.tile([B, K], F32)
            nc.scalar.activation(e2[:], a1[:], AF.Exp, bias=m2ns[:, 0:1], scale=1.0 / TAU)
            s2 = const.tile([B, 1], F32)
            nc.vector.tensor_reduce(s2[:], e2[:], axis=AX.X, op=ALU.add)
            r2 = const.tile([B, 1], F32)
            nc.vector.reciprocal(r2[:], s2[:])
            attn = const.tile([B, K], F32R)
            nc.vector.tensor_scalar_mul(attn[:], e2[:], r2[:, 0:1])

            # attn [j, k] -> attn_T [k, j] via PE transpose
            tps = psM.tile([K, B], F32R, tag="mlp")
            nc.tensor.transpose(tps[:], attn[:], id8[:])
            atT = const.tile([K, B], F32R)
            nc.scalar.copy(atT[:], tps[:])

            # block-diagonal blend weights: BD[c*4+k, j*16+c] = attn[j, k],
            # replicated in both partition halves so the lhsT slice's
            # base_partition can match the rhs slice's (matmul requirement)
            BD = const.tile([128, 128], F32R)
            nc.vector.tensor_copy(BD[:], ztile[:])
            BDv = BD[:].rearrange("p (j c) -> p j c", c=16)
            for half in range(2):
                for c in range(16):
                    # compute engines can't start at partition 4c; DMA can
                    p0 = 64 * half + c * 4
                    nc.sync.dma_start(BDv[p0 : p0 + 4, :, c], atT[:])

            # interleave remaining convs with blends so output DMA overlaps compute
            emit_blend(0, BD)
            emit_blend(1, BD)
            emit_blend(2, BD)
            for t in range(3, 8):
                emit_conv(t)
                emit_blend(t, BD)

    nc.compile()
    return nc


def pack_inputs(x, conv_w, conv_b, w1, b1, w2, b2):
    """Host-side layout packing (no arithmetic beyond constant folding of the
    mean-pool scale into w1)."""
    x = np.ascontiguousarray(x, dtype=np.float32)
    x_all = x.reshape(B, CIN, HW2)

    # conv_w [K, COUT, CIN, 3, 3] -> [ci, t, tap, p] with p = c*4 + k,
    # co = 32 t + c
    w = np.asarray(conv_w, dtype=np.float32).transpose(2, 3, 4, 0, 1)  # ci kh kw k co
    w = w.reshape(CIN, KS, KS, K, 8, 32)  # ci kh kw k t c
    w = w.transpose(0, 4, 1, 2, 5, 3)  # ci t kh kw c k
    wconv = np.ascontiguousarray(w.reshape(CIN, 8 * 9 * 128))

    bc = np.asarray(conv_b, dtype=np.float32).reshape(K, 8, 32)  # k t c
    bconv = np.ascontiguousarray(bc.transpose(1, 2, 0).reshape(8, 128).T)  # [p, t]

    w1t = np.ascontiguousarray(np.asarray(w1, dtype=np.float32).T) / float(HW2)
    b1c = np.ascontiguousarray(np.asarray(b1, dtype=np.float32).reshape(2, 128).T)
    w2T = np.asarray(w2, dtype=np.float32).T  # [256, 4]
    w2t = np.ascontiguousarray(np.concatenate([w2T[:128], w2T[128:]], axis=1))
    b2r = np.asarray(b2, dtype=np.float32).reshape(1, K)
    ident8 = np.eye(B, dtype=np.float32)

    common = dict(
        x_all=x_all, wconv=wconv, bconv=bconv, w1t=w1t, b1c=b1c,
        w2t=w2t, b2r=b2r, ident8=ident8,
        zer128=np.zeros((128, 128), dtype=np.float32),
        one18=np.ones((1, B), dtype=np.float32),
    )
    in_maps = [dict(common, xi=np.ascontiguousarray(x_all[i])) for i in range(NCORES)]
    return in_maps


def run(inputs, trace=False):
    from concourse.bass_utils import run_bass_kernel_spmd

    nc = build_nc()
    in_maps = pack_inputs(**inputs)
    res = run_bass_kernel_spmd(
        nc, in_maps, core_ids=list(range(NCORES)), trace=trace
    )
    slabs = [res.results[i]["out"] for i in range(NCORES)]
    out = np.stack(slabs, axis=0).reshape(B, B, COUT, HW, HW)
    return out, res


def kernel(**inputs) -> np.ndarray:
    out, _ = run(inputs, trace=False)
    return out
